# revision 3
# baseline (speedup 1.0000x reference)
"""Multi-head self-attention (RoPE, eval-mode) Trainium2 Bass kernel.

Problem: B=2, T=2048, D=1024, H=16, d_head=64, fp32 I/O.

Sharding (8 cores): core c handles batch b=c//4 and the 4 heads
[4g, 4g+4) where g=c%4.  QKV/attention are head-local; the output
projection produces a per-core partial (contraction over this core's
256 head-dims) which the host sums across the 4 cores of each batch
and adds b_out.

Per-core design notes:
  - All matmul operands are bf16 (PSUM accumulation stays fp32): same
    PE rate as fp32r but halves input DMA and SBUF footprint, and
    LDWEIGHTS drops to ~95ns.  Measured rel-err ~7e-3 (gate: 2e-2).
  - q,k are computed feature-major (d_head on partitions, T on free);
    2 heads stacked per 128-partition tile.
  - RoPE: rotate_half is a 32-partition block swap (SBUF->SBUF DMAs)
    with the sign folded into the host-provided sin table; one mul on
    DVE, one on GpSimd so DVE doesn't bottleneck the QKV phase.
  - k is roped into per-head zero-padded kpad tiles (all-K=128 matmuls
    keep HAM from throttling the PE clock).
  - v is stored per head as [ones | v] 128-wide stationary tiles, so
    each PV matmul yields the softmax denominators (partitions 0:64)
    and attn^T (64:128) in one pass.
  - softmax skips max-subtraction and normalizes after PV with the
    fast DVE reciprocal.
  - attention is exp-bound (~1.11us per tk tile on the ACT engine);
    outproj matmuls are spread ONE per tk so per-tk PE work stays just
    under the exp time and the ACT engine never waits.
  - DMA priority: the first fm chain's deps (w_qk + x quarter 0) are
    issued first, chunked per 128-feature block so the PE starts ~9us
    in; later x quarters/wv/cos/sin/w_o trail behind.
  - trailing outproj units (last tq block) use the score-PSUM pool
    (free by then) and DMA out via sync/scalar queues so the gpsimd
    drain at teardown has nothing to wait on.
"""

import numpy as np
import ml_dtypes

B, T, D = 2, 2048, 1024
H = 16
DH = 64
NCORES = 8
P = 128

BF16 = ml_dtypes.bfloat16

_CACHE = {}


def _rope_tables_np():
    theta = 1.0 / (10000.0 ** (np.arange(0, DH, 2, dtype=np.float64) / DH))
    angles = np.outer(np.arange(T, dtype=np.float64), theta)  # (T, 32)
    angles = np.concatenate([angles, angles], axis=-1)  # (T, DH)
    cos = np.cos(angles).astype(np.float32)
    sin = np.sin(angles).astype(np.float32)
    cosT = np.ascontiguousarray(cos.T)  # (64, T)
    sinT = np.ascontiguousarray(sin.T)
    sinT_signed = np.concatenate([-sinT[0:32], sinT[32:64]], axis=0)
    cos2 = np.tile(cosT, (2, 1))  # (128, T)
    sin2 = np.tile(sinT_signed, (2, 1))
    return cos2.astype(BF16), sin2.astype(BF16)


def _build_module():
    import concourse.mybir as mybir
    import concourse.tile as tile
    from concourse import bacc

    f32 = mybir.dt.float32
    bf16 = mybir.dt.bfloat16

    nc = bacc.Bacc("TRN2", target_bir_lowering=False, debug=False)
    xT = nc.dram_tensor("xT", [4, P, 8, 512], bf16, kind="ExternalInput")
    w_qk = nc.dram_tensor("w_qk", [P, 8, 512], bf16, kind="ExternalInput")
    w_v = nc.dram_tensor("w_v", [P, 8, 256], bf16, kind="ExternalInput")
    w_o = nc.dram_tensor("w_o", [P, 2, 1024], bf16, kind="ExternalInput")
    cos2 = nc.dram_tensor("cos2", [P, T], bf16, kind="ExternalInput")
    sin2 = nc.dram_tensor("sin2", [P, T], bf16, kind="ExternalInput")
    out = nc.dram_tensor("out", [T, D], bf16, kind="ExternalOutput")

    Exp = mybir.ActivationFunctionType.Exp

    with tile.TileContext(nc) as tc:
        with tc.tile_pool(name="persist", bufs=1) as persist:
            wqk_sb = [
                persist.tile([P, 4, 512], bf16, tag=f"wqk{i}", name=f"wqk{i}")
                for i in range(2)
            ]
            wv_sb = persist.tile([P, 8, 256], bf16)
            q_q = [
                [
                    persist.tile([P, 512], bf16, tag=f"q{hp}_{q}", name=f"q{hp}_{q}")
                    for q in range(4)
                ]
                for hp in range(2)
            ]
            kpad = [
                [
                    [
                        persist.tile(
                            [P, 512], bf16, tag=f"kp{hp}{h}_{q}", name=f"kp{hp}{h}_{q}"
                        )
                        for q in range(4)
                    ]
                    for h in range(2)
                ]
                for hp in range(2)
            ]
            # per (tk-tile, head): [ones | v] stationary 128x128
            vaug = persist.tile([P, 16, 4, P], bf16)

            with (
                tc.tile_pool(name="attnsb", bufs=1) as apool,
                tc.tile_pool(name="expp", bufs=3) as epool,
                tc.tile_pool(name="norm", bufs=1) as npool,
            ):
                attn_q = [
                    [
                        apool.tile(
                            [P, 512], bf16, tag=f"at{hp}_{b}", name=f"at{hp}_{b}"
                        )
                        for b in range(4)
                    ]
                    for hp in range(2)
                ]

                with (
                    tc.tile_pool(name="xt", bufs=2) as xpool,
                    tc.tile_pool(name="kst", bufs=1) as kpool,
                    tc.tile_pool(name="qkv_ps", bufs=2, space="PSUM") as qkps,
                    tc.tile_pool(name="rope", bufs=2) as rpool,
                ):
                    cos_sb = kpool.tile([P, T], bf16)
                    sin_sb = kpool.tile([P, T], bf16)
                    kstack = [
                        [
                            kpool.tile(
                                [P, 512], bf16, tag=f"ks{hp}_{q}", name=f"ks{hp}_{q}"
                            )
                            for q in range(4)
                        ]
                        for hp in range(2)
                    ]

                    # ---- input DMAs, priority-ordered ----------------------
                    # First fm chain needs wqk chunk j + x chunk j in dc
                    # order; chunked DMAs let the chain start after ~256KB
                    # instead of 2MB.  Scalar queue: weights; sync: x.
                    xts = []
                    for tq in range(4):
                        xtl = xpool.tile([P, 4, 512], bf16, tag="xtl", name="xtl")
                        xth = xpool.tile([P, 4, 512], bf16, tag="xth", name="xth")
                        xts.append((xtl, xth))
                    for j in range(4):
                        nc.scalar.dma_start(wqk_sb[0][:, j, :], w_qk[:, j, :])
                        nc.sync.dma_start(xts[0][0][:, j, :], xT[0, :, j, :])
                    for j in range(4):
                        nc.scalar.dma_start(wqk_sb[1][:, j, :], w_qk[:, 4 + j, :])
                        nc.sync.dma_start(xts[0][1][:, j, :], xT[0, :, 4 + j, :])
                    for tq in range(1, 4):
                        nc.sync.dma_start(xts[tq][0][:], xT[tq, :, 0:4, :])
                        nc.sync.dma_start(xts[tq][1][:], xT[tq, :, 4:8, :])
                    nc.scalar.dma_start(wv_sb[:], w_v[:])
                    nc.scalar.dma_start(cos_sb[:], cos2[:])
                    nc.scalar.dma_start(sin_sb[:], sin2[:])

                    # Memsets early: no deps, run during the DMA window.
                    for hp in range(2):
                        for q in range(4):
                            nc.vector.memset(
                                kpad[hp][0][q][64:128, :].bitcast(f32), 0.0
                            )
                            nc.vector.memset(
                                kpad[hp][1][q][0:64, :].bitcast(f32), 0.0
                            )
                    # bf16 1.0 pair as an f32 bit pattern (0x3F803F80).
                    ones_f32 = float(np.uint32(0x3F803F80).view(np.float32))
                    nc.vector.memset(
                        vaug[:, :, :, 0:64].bitcast(f32), ones_f32
                    )

                    def fm_dst(cc, tq):
                        return (q_q if cc in (0, 2) else kstack)[cc // 2][tq]

                    def fm_chain(xt, tq, cc):
                        """One feature-major QKV chain (q or stacked k)."""
                        ps = qkps.tile([P, 512], f32, tag="fm", name="fmps")
                        for dc in range(8):
                            nc.tensor.matmul(
                                ps[:],
                                lhsT=wqk_sb[dc // 4][:, dc % 4, cc * P : (cc + 1) * P],
                                rhs=xt[dc // 4][:, dc % 4, :],
                                start=(dc == 0),
                                stop=(dc == 7),
                            )
                        # PSUM->SBUF copy on ACT (idle during QKV); DVE is
                        # busy with rope muls/adds.
                        nc.scalar.copy(fm_dst(cc, tq)[:], ps[:])

                    def v_chain(xt, tq, t4):
                        psv = qkps.tile([P, 256], f32, tag="v", name="vps")
                        for dc in range(8):
                            nc.tensor.matmul(
                                psv[:],
                                lhsT=xt[dc // 4][:, dc % 4, t4 * P : (t4 + 1) * P],
                                rhs=wv_sb[:, dc, :],
                                start=(dc == 0),
                                stop=(dc == 7),
                            )
                        tki = tq * 4 + t4
                        nc.scalar.copy(
                            vaug[:, tki, :, 64:128],
                            psv.rearrange("p (h e) -> p h e", e=64),
                        )

                    def rope_q(cc, qtr):
                        """RoPE one T-quarter of one q/k tensor."""
                        base = fm_dst(cc, qtr)
                        hs = slice(qtr * 512, (qtr + 1) * 512)
                        rot = rpool.tile([P, 512], bf16, tag="rot", name="rot")
                        for blk in range(4):
                            s = (blk ^ 1) * 32
                            eng = nc.sync if blk % 2 == 0 else nc.gpsimd
                            eng.dma_start(
                                rot[blk * 32 : (blk + 1) * 32, :],
                                base[s : s + 32, :],
                            )
                        t1 = rpool.tile([P, 512], bf16, tag="t1", name="t1")
                        nc.vector.tensor_mul(t1[:], base[:], cos_sb[:, hs])
                        nc.gpsimd.tensor_mul(rot[:], rot[:], sin_sb[:, hs])
                        if cc in (0, 2):
                            nc.vector.tensor_add(base[:], t1[:], rot[:])
                        else:
                            hp = cc // 2
                            nc.vector.tensor_add(
                                kpad[hp][0][qtr][0:64, :], t1[0:64, :], rot[0:64, :]
                            )
                            nc.vector.tensor_add(
                                kpad[hp][1][qtr][64:128, :],
                                t1[64:128, :],
                                rot[64:128, :],
                            )

                    for tq in range(4):
                        for cc in (1, 0, 3, 2):
                            fm_chain(xts[tq], tq, cc)
                        for t4 in range(4):
                            v_chain(xts[tq], tq, t4)
                        for cc in (1, 0, 3, 2):
                            rope_q(cc, tq)

                # ---- attention + interleaved output projection ----------
                # PSUM: sc 4 + pv 2 + po 2 = 8 banks.
                with (
                    tc.tile_pool(name="wop", bufs=1) as wpool,
                    tc.tile_pool(name="ob", bufs=3) as opool,
                    tc.tile_pool(name="sc_ps", bufs=2, space="PSUM") as scps,
                    tc.tile_pool(name="pv_ps", bufs=1, space="PSUM") as pvps,
                    tc.tile_pool(name="po_ps", bufs=2, space="PSUM") as pops,
                ):
                    wo_sb = wpool.tile([P, 2, 1024], bf16)
                    nc.scalar.dma_start(wo_sb[:], w_o[:])

                    # Interleaved outproj, ONE matmul per tk: step s of 16
                    # emits (tqc, d2, hp) = (s//4, (s%4)//2, s%2) for block b.
                    # po tile [128,512] per (tqc,d2); cast+DMA after hp=1.
                    po_cur = [None]

                    def outproj_step(b, s):
                        tqc, d2, hp = s // 4, (s % 4) // 2, s % 2
                        row = b * 4 + tqc
                        if hp == 0:
                            po_cur[0] = pops.tile([P, 512], f32, tag="po", name="po")
                        po = po_cur[0]
                        nc.tensor.matmul(
                            po[:],
                            lhsT=attn_q[hp][b][:, tqc * P : (tqc + 1) * P],
                            rhs=wo_sb[:, hp, d2 * 512 : (d2 + 1) * 512],
                            start=(hp == 0),
                            stop=(hp == 1),
                        )
                        if hp == 1:
                            ob = opool.tile([P, 512], bf16, tag="ob", name="ob")
                            nc.vector.tensor_copy(ob[:], po[:])
                            seng = nc.sync if d2 == 0 else nc.gpsimd
                            seng.dma_start(
                                out[row * P : (row + 1) * P, d2 * 512 : (d2 + 1) * 512],
                                ob[:],
                            )

                    def outproj_unit_tail(b, tqc):
                        """Trailing unit: po lives in the (now free) score
                        PSUM pool; one cast + one DMA; sync/scalar queues so
                        the gpsimd teardown drain has nothing to wait on."""
                        row = b * 4 + tqc
                        po = scps.tile([P, 1024], f32, tag="sc", name="sc")
                        for d2 in range(2):
                            for hp in range(2):
                                nc.tensor.matmul(
                                    po[:, d2 * 512 : (d2 + 1) * 512],
                                    lhsT=attn_q[hp][b][:, tqc * P : (tqc + 1) * P],
                                    rhs=wo_sb[:, hp, d2 * 512 : (d2 + 1) * 512],
                                    start=(hp == 0),
                                    stop=(hp == 1),
                                )
                        ob = opool.tile([P, 1024], bf16, tag="obt", name="obt")
                        nc.vector.tensor_copy(ob[:], po[:])
                        eng = nc.sync if tqc % 2 == 0 else nc.scalar
                        eng.dma_start(out[row * P : (row + 1) * P, :], ob[:])

                    for hp in range(2):
                        for tq in range(4):  # tq blocks of 512
                            prev_b = tq - 1 if (hp == 1 and tq > 0) else None
                            pv = [
                                pvps.tile([P, 512], f32, tag=f"pv{h}", name=f"pv{h}")
                                for h in range(2)
                            ]
                            for tk in range(16):
                                if prev_b is not None:
                                    outproj_step(prev_b, tk)
                                sc = scps.tile([P, 1024], f32, tag="sc", name="sc")
                                ko = (tk % 4) * P
                                for h in range(2):
                                    nc.tensor.matmul(
                                        sc[:, h * 512 : (h + 1) * 512],
                                        lhsT=kpad[hp][h][tk // 4][:, ko : ko + P],
                                        rhs=q_q[hp][tq][:],
                                        start=True,
                                        stop=True,
                                    )
                                ex = epool.tile([P, 1024], bf16, tag="e", name="e")
                                nc.scalar.activation(ex[:], sc[:], Exp, scale=0.125)
                                for h in range(2):
                                    nc.tensor.matmul(
                                        pv[h][:],
                                        lhsT=vaug[:, tk, hp * 2 + h, :],
                                        rhs=ex[:, h * 512 : (h + 1) * 512],
                                        start=(tk == 0),
                                        stop=(tk == 15),
                                    )
                            for h in range(2):
                                rc = npool.tile([64, 512], f32, tag="rc", name="rc")
                                nc.vector.reciprocal_approx_fast(
                                    rc[:], pv[h][0:64, :]
                                )
                                hb = h * 64
                                nc.vector.tensor_mul(
                                    attn_q[hp][tq][hb : hb + 64, :],
                                    pv[h][64:128, :],
                                    rc[:],
                                )
                    for tqc in range(4):
                        outproj_unit_tail(3, tqc)

    nc.compile()
    return nc


def _get_module():
    if "nc" not in _CACHE:
        _CACHE["nc"] = _build_module()
    return _CACHE["nc"]


def make_in_maps(x, w_qkv, w_out):
    cos2, sin2 = _rope_tables_np()
    xb = x.astype(BF16)
    wq = w_qkv.astype(BF16)
    wo = w_out.astype(BF16)
    in_maps = []
    for c in range(NCORES):
        b, g = divmod(c, 4)
        q0 = 256 * g
        # column chunks: [q_hp0 | k_hp0 | q_hp1 | k_hp1]
        wqk_c = np.concatenate(
            [
                wq[:, q0 : q0 + 128],
                wq[:, 1024 + q0 : 1024 + q0 + 128],
                wq[:, q0 + 128 : q0 + 256],
                wq[:, 1024 + q0 + 128 : 1024 + q0 + 256],
            ],
            axis=1,
        )
        xt4 = np.ascontiguousarray(
            xb[b].T.reshape(8, 128, 4, 512).transpose(2, 1, 0, 3)
        )
        wv_c = wq[:, 2048 + q0 : 2048 + q0 + 256]
        in_maps.append(
            {
                "xT": xt4,
                "w_qk": np.ascontiguousarray(
                    wqk_c.reshape(8, 128, 512).transpose(1, 0, 2)
                ),
                "w_v": np.ascontiguousarray(
                    wv_c.reshape(8, 128, 256).transpose(1, 0, 2)
                ),
                "w_o": np.ascontiguousarray(
                    wo[q0 : q0 + 256, :].reshape(2, 128, 1024).transpose(1, 0, 2)
                ),
                "cos2": cos2,
                "sin2": sin2,
            }
        )
    return in_maps


def combine_outputs(results, b_out):
    out = np.empty((B, T, D), dtype=np.float32)
    for b in range(B):
        acc = results[4 * b]["out"].astype(np.float32)
        for c in range(4 * b + 1, 4 * b + 4):
            acc = acc + results[c]["out"].astype(np.float32)
        out[b] = acc + b_out[None, :]
    return out


def kernel(x, w_qkv, w_out, b_out, _trace=False, _tag=[0]):
    from concourse import bass_utils

    nc = _get_module()
    in_maps = make_in_maps(
        np.asarray(x, dtype=np.float32),
        np.asarray(w_qkv, dtype=np.float32),
        np.asarray(w_out, dtype=np.float32),
    )
    res = bass_utils.run_bass_kernel_spmd(
        nc, in_maps, core_ids=list(range(NCORES)), trace=_trace
    )
    if _trace:
        _CACHE["last_result"] = res
    return combine_outputs(res.results, np.asarray(b_out, dtype=np.float32))


# revision 5
# speedup vs baseline: 1.0554x; 1.0554x over previous
"""Multi-head self-attention (RoPE, eval-mode) Trainium2 Bass kernel.

Problem: B=2, T=2048, D=1024, H=16, d_head=64, fp32 I/O.

Sharding (8 cores): core c handles batch b=c//4 and the 4 heads
[4g, 4g+4) where g=c%4.  QKV/attention are head-local; the output
projection produces a per-core partial (contraction over this core's
256 head-dims) which the host sums across the 4 cores of each batch
and adds b_out.

Per-core design notes:
  - All matmul operands are bf16 (PSUM accumulation stays fp32): same
    PE rate as fp32r but halves input DMA and SBUF footprint, and
    LDWEIGHTS drops to ~95ns.  Measured rel-err ~7e-3 (gate: 2e-2).
  - q,k are computed feature-major (d_head on partitions, T on free);
    2 heads stacked per 128-partition tile.
  - RoPE: rotate_half is a 32-partition block swap (SBUF->SBUF DMAs)
    with the sign folded into the host-provided sin table; one mul on
    DVE, one on GpSimd so DVE doesn't bottleneck the QKV phase.
  - k is roped into per-head zero-padded kpad tiles (all-K=128 matmuls
    keep HAM from throttling the PE clock).
  - v is stored per head as [ones | v] 128-wide stationary tiles, so
    each PV matmul yields the softmax denominators (partitions 0:64)
    and attn^T (64:128) in one pass.
  - softmax skips max-subtraction and normalizes after PV with the
    fast DVE reciprocal.
  - attention is exp-bound (~1.11us per tk tile on the ACT engine);
    outproj matmuls are spread ONE per tk so per-tk PE work stays just
    under the exp time and the ACT engine never waits.
  - DMA priority: the first fm chain's deps (w_qk + x quarter 0) are
    issued first, chunked per 128-feature block so the PE starts ~9us
    in; later x quarters/wv/cos/sin/w_o trail behind.
  - trailing outproj units (last tq block) use the score-PSUM pool
    (free by then) and DMA out via sync/scalar queues so the gpsimd
    drain at teardown has nothing to wait on.
"""

import numpy as np
import ml_dtypes

B, T, D = 2, 2048, 1024
H = 16
DH = 64
NCORES = 8
P = 128

BF16 = ml_dtypes.bfloat16

_CACHE = {}


def _rope_tables_np():
    theta = 1.0 / (10000.0 ** (np.arange(0, DH, 2, dtype=np.float64) / DH))
    angles = np.outer(np.arange(T, dtype=np.float64), theta)  # (T, 32)
    angles = np.concatenate([angles, angles], axis=-1)  # (T, DH)
    cos = np.cos(angles).astype(np.float32)
    sin = np.sin(angles).astype(np.float32)
    cosT = np.ascontiguousarray(cos.T)  # (64, T)
    sinT = np.ascontiguousarray(sin.T)
    sinT_signed = np.concatenate([-sinT[0:32], sinT[32:64]], axis=0)
    cos2 = np.tile(cosT, (2, 1))  # (128, T)
    sin2 = np.tile(sinT_signed, (2, 1))
    return cos2.astype(BF16), sin2.astype(BF16)


def _build_module():
    import concourse.mybir as mybir
    import concourse.tile as tile
    from concourse import bacc

    f32 = mybir.dt.float32
    bf16 = mybir.dt.bfloat16

    nc = bacc.Bacc("TRN2", target_bir_lowering=False, debug=False)
    xT = nc.dram_tensor("xT", [4, P, 8, 512], bf16, kind="ExternalInput")
    w_qk = nc.dram_tensor("w_qk", [P, 8, 512], bf16, kind="ExternalInput")
    w_v = nc.dram_tensor("w_v", [P, 8, 256], bf16, kind="ExternalInput")
    w_o = nc.dram_tensor("w_o", [P, 2, 1024], bf16, kind="ExternalInput")
    cos2 = nc.dram_tensor("cos2", [P, T], bf16, kind="ExternalInput")
    sin2 = nc.dram_tensor("sin2", [P, T], bf16, kind="ExternalInput")
    out = nc.dram_tensor("out", [T, D], bf16, kind="ExternalOutput")

    Exp = mybir.ActivationFunctionType.Exp

    with tile.TileContext(nc) as tc:
        with tc.tile_pool(name="persist", bufs=1) as persist:
            wqk_sb = [
                persist.tile([P, 4, 512], bf16, tag=f"wqk{i}", name=f"wqk{i}")
                for i in range(2)
            ]
            wv_sb = persist.tile([P, 8, 256], bf16)
            q_q = [
                [
                    persist.tile([P, 512], bf16, tag=f"q{hp}_{q}", name=f"q{hp}_{q}")
                    for q in range(4)
                ]
                for hp in range(2)
            ]
            kpad = [
                [
                    [
                        persist.tile(
                            [P, 512], bf16, tag=f"kp{hp}{h}_{q}", name=f"kp{hp}{h}_{q}"
                        )
                        for q in range(4)
                    ]
                    for h in range(2)
                ]
                for hp in range(2)
            ]
            # per (tk-tile, head): [ones | v] stationary 128x128
            vaug = persist.tile([P, 16, 4, P], bf16)

            with (
                tc.tile_pool(name="attnsb", bufs=1) as apool,
                tc.tile_pool(name="expp", bufs=3) as epool,
                tc.tile_pool(name="norm", bufs=1) as npool,
            ):
                attn_q = [
                    [
                        apool.tile(
                            [P, 512], bf16, tag=f"at{hp}_{b}", name=f"at{hp}_{b}"
                        )
                        for b in range(4)
                    ]
                    for hp in range(2)
                ]

                with (
                    tc.tile_pool(name="xt", bufs=2) as xpool,
                    tc.tile_pool(name="kst", bufs=1) as kpool,
                    tc.tile_pool(name="qkv_ps", bufs=2, space="PSUM") as qkps,
                    tc.tile_pool(name="rope", bufs=2) as rpool,
                ):
                    cos_sb = kpool.tile([P, T], bf16)
                    sin_sb = kpool.tile([P, T], bf16)
                    kstack = [
                        [
                            kpool.tile(
                                [P, 512], bf16, tag=f"ks{hp}_{q}", name=f"ks{hp}_{q}"
                            )
                            for q in range(4)
                        ]
                        for hp in range(2)
                    ]

                    # ---- input DMAs, priority-ordered ----------------------
                    # First fm chain needs wqk chunk j + x chunk j in dc
                    # order; chunked DMAs let the chain start after ~256KB
                    # instead of 2MB.  Scalar queue: weights; sync: x.
                    xts = []
                    for tq in range(4):
                        xtl = xpool.tile([P, 4, 512], bf16, tag="xtl", name="xtl")
                        xth = xpool.tile([P, 4, 512], bf16, tag="xth", name="xth")
                        xts.append((xtl, xth))
                    # 2-chunk split: finer would be DMA-issue-rate limited
                    # (~0.85us per issue per queue) and starve the chain.
                    for j in range(2):
                        jj = slice(2 * j, 2 * j + 2)
                        nc.scalar.dma_start(wqk_sb[0][:, jj, :], w_qk[:, jj, :])
                        nc.sync.dma_start(xts[0][0][:, jj, :], xT[0, :, jj, :])
                    for j in range(2):
                        jj = slice(2 * j, 2 * j + 2)
                        jh = slice(4 + 2 * j, 4 + 2 * j + 2)
                        nc.scalar.dma_start(wqk_sb[1][:, jj, :], w_qk[:, jh, :])
                        nc.sync.dma_start(xts[0][1][:, jj, :], xT[0, :, jh, :])
                    for tq in range(1, 4):
                        nc.sync.dma_start(xts[tq][0][:], xT[tq, :, 0:4, :])
                        nc.sync.dma_start(xts[tq][1][:], xT[tq, :, 4:8, :])
                    nc.scalar.dma_start(wv_sb[:], w_v[:])
                    nc.scalar.dma_start(cos_sb[:], cos2[:])
                    nc.scalar.dma_start(sin_sb[:], sin2[:])

                    # Memsets early: no deps, run during the DMA window.
                    for hp in range(2):
                        for q in range(4):
                            nc.vector.memset(
                                kpad[hp][0][q][64:128, :].bitcast(f32), 0.0
                            )
                            nc.vector.memset(
                                kpad[hp][1][q][0:64, :].bitcast(f32), 0.0
                            )
                    # bf16 1.0 pair as an f32 bit pattern (0x3F803F80).
                    ones_f32 = float(np.uint32(0x3F803F80).view(np.float32))
                    nc.vector.memset(
                        vaug[:, :, :, 0:64].bitcast(f32), ones_f32
                    )

                    def fm_dst(cc, tq):
                        return (q_q if cc in (0, 2) else kstack)[cc // 2][tq]

                    def fm_chain(xt, tq, cc):
                        """One feature-major QKV chain (q or stacked k)."""
                        ps = qkps.tile([P, 512], f32, tag="fm", name="fmps")
                        for dc in range(8):
                            nc.tensor.matmul(
                                ps[:],
                                lhsT=wqk_sb[dc // 4][:, dc % 4, cc * P : (cc + 1) * P],
                                rhs=xt[dc // 4][:, dc % 4, :],
                                start=(dc == 0),
                                stop=(dc == 7),
                            )
                        # PSUM->SBUF copy on ACT (idle during QKV); DVE is
                        # busy with rope muls/adds.
                        nc.scalar.copy(fm_dst(cc, tq)[:], ps[:])

                    def v_chain(xt, tq, t4):
                        psv = qkps.tile([P, 256], f32, tag="v", name="vps")
                        for dc in range(8):
                            nc.tensor.matmul(
                                psv[:],
                                lhsT=xt[dc // 4][:, dc % 4, t4 * P : (t4 + 1) * P],
                                rhs=wv_sb[:, dc, :],
                                start=(dc == 0),
                                stop=(dc == 7),
                            )
                        tki = tq * 4 + t4
                        nc.scalar.copy(
                            vaug[:, tki, :, 64:128],
                            psv.rearrange("p (h e) -> p h e", e=64),
                        )

                    def rope_q(cc, qtr):
                        """RoPE one T-quarter of one q/k tensor."""
                        base = fm_dst(cc, qtr)
                        hs = slice(qtr * 512, (qtr + 1) * 512)
                        rot = rpool.tile([P, 512], bf16, tag="rot", name="rot")
                        for blk in range(4):
                            s = (blk ^ 1) * 32
                            eng = nc.sync if blk % 2 == 0 else nc.gpsimd
                            eng.dma_start(
                                rot[blk * 32 : (blk + 1) * 32, :],
                                base[s : s + 32, :],
                            )
                        t1 = rpool.tile([P, 512], bf16, tag="t1", name="t1")
                        nc.vector.tensor_mul(t1[:], base[:], cos_sb[:, hs])
                        nc.vector.tensor_mul(rot[:], rot[:], sin_sb[:, hs])
                        if cc in (0, 2):
                            nc.vector.tensor_add(base[:], t1[:], rot[:])
                        else:
                            hp = cc // 2
                            nc.vector.tensor_add(
                                kpad[hp][0][qtr][0:64, :], t1[0:64, :], rot[0:64, :]
                            )
                            nc.vector.tensor_add(
                                kpad[hp][1][qtr][64:128, :],
                                t1[64:128, :],
                                rot[64:128, :],
                            )

                    for tq in range(4):
                        for cc in (1, 0, 3, 2):
                            fm_chain(xts[tq], tq, cc)
                        for t4 in range(4):
                            v_chain(xts[tq], tq, t4)
                        for cc in (1, 0, 3, 2):
                            rope_q(cc, tq)

                # ---- attention + interleaved output projection ----------
                # PSUM: sc 4 + pv 2 + po 2 = 8 banks.
                with (
                    tc.tile_pool(name="wop", bufs=1) as wpool,
                    tc.tile_pool(name="ob", bufs=3) as opool,
                    tc.tile_pool(name="sc_ps", bufs=2, space="PSUM") as scps,
                    tc.tile_pool(name="pv_ps", bufs=1, space="PSUM") as pvps,
                    tc.tile_pool(name="po_ps", bufs=2, space="PSUM") as pops,
                ):
                    wo_sb = wpool.tile([P, 2, 1024], bf16)
                    nc.scalar.dma_start(wo_sb[:], w_o[:])

                    # Interleaved outproj, ONE matmul per tk: step s of 16
                    # emits (tqc, d2, hp) = (s//4, (s%4)//2, s%2) for block b.
                    # po tile [128,512] per (tqc,d2); cast+DMA after hp=1.
                    po_cur = [None]

                    def outproj_step(b, s):
                        tqc, d2, hp = s // 4, (s % 4) // 2, s % 2
                        row = b * 4 + tqc
                        if hp == 0:
                            po_cur[0] = pops.tile([P, 512], f32, tag="po", name="po")
                        po = po_cur[0]
                        nc.tensor.matmul(
                            po[:],
                            lhsT=attn_q[hp][b][:, tqc * P : (tqc + 1) * P],
                            rhs=wo_sb[:, hp, d2 * 512 : (d2 + 1) * 512],
                            start=(hp == 0),
                            stop=(hp == 1),
                        )
                        if hp == 1:
                            ob = opool.tile([P, 512], bf16, tag="ob", name="ob")
                            nc.vector.tensor_copy(ob[:], po[:])
                            seng = nc.sync if d2 == 0 else nc.gpsimd
                            seng.dma_start(
                                out[row * P : (row + 1) * P, d2 * 512 : (d2 + 1) * 512],
                                ob[:],
                            )

                    def outproj_unit_tail(b, tqc):
                        """Trailing unit: po lives in the (now free) score
                        PSUM pool; one cast + one DMA; sync/scalar queues so
                        the gpsimd teardown drain has nothing to wait on."""
                        row = b * 4 + tqc
                        po = scps.tile([P, 1024], f32, tag="sc", name="sc")
                        for d2 in range(2):
                            for hp in range(2):
                                nc.tensor.matmul(
                                    po[:, d2 * 512 : (d2 + 1) * 512],
                                    lhsT=attn_q[hp][b][:, tqc * P : (tqc + 1) * P],
                                    rhs=wo_sb[:, hp, d2 * 512 : (d2 + 1) * 512],
                                    start=(hp == 0),
                                    stop=(hp == 1),
                                )
                        ob = opool.tile([P, 1024], bf16, tag="obt", name="obt")
                        nc.vector.tensor_copy(ob[:], po[:])
                        eng = nc.sync if tqc % 2 == 0 else nc.scalar
                        eng.dma_start(out[row * P : (row + 1) * P, :], ob[:])

                    for hp in range(2):
                        for tq in range(4):  # tq blocks of 512
                            prev_b = tq - 1 if (hp == 1 and tq > 0) else None
                            pv = [
                                pvps.tile([P, 512], f32, tag=f"pv{h}", name=f"pv{h}")
                                for h in range(2)
                            ]
                            for tk in range(16):
                                if prev_b is not None:
                                    outproj_step(prev_b, tk)
                                sc = scps.tile([P, 1024], f32, tag="sc", name="sc")
                                ko = (tk % 4) * P
                                for h in range(2):
                                    nc.tensor.matmul(
                                        sc[:, h * 512 : (h + 1) * 512],
                                        lhsT=kpad[hp][h][tk // 4][:, ko : ko + P],
                                        rhs=q_q[hp][tq][:],
                                        start=True,
                                        stop=True,
                                    )
                                ex = epool.tile([P, 1024], bf16, tag="e", name="e")
                                nc.scalar.activation(ex[:], sc[:], Exp, scale=0.125)
                                for h in range(2):
                                    nc.tensor.matmul(
                                        pv[h][:],
                                        lhsT=vaug[:, tk, hp * 2 + h, :],
                                        rhs=ex[:, h * 512 : (h + 1) * 512],
                                        start=(tk == 0),
                                        stop=(tk == 15),
                                    )
                            for h in range(2):
                                rc = npool.tile([64, 512], f32, tag="rc", name="rc")
                                nc.vector.reciprocal_approx_fast(
                                    rc[:], pv[h][0:64, :]
                                )
                                hb = h * 64
                                nc.vector.tensor_mul(
                                    attn_q[hp][tq][hb : hb + 64, :],
                                    pv[h][64:128, :],
                                    rc[:],
                                )
                    for tqc in range(4):
                        outproj_unit_tail(3, tqc)

    nc.compile()
    return nc


def _get_module():
    if "nc" not in _CACHE:
        _CACHE["nc"] = _build_module()
    return _CACHE["nc"]


def make_in_maps(x, w_qkv, w_out):
    cos2, sin2 = _rope_tables_np()
    xb = x.astype(BF16)
    wq = w_qkv.astype(BF16)
    wo = w_out.astype(BF16)
    in_maps = []
    for c in range(NCORES):
        b, g = divmod(c, 4)
        q0 = 256 * g
        # column chunks: [q_hp0 | k_hp0 | q_hp1 | k_hp1]
        wqk_c = np.concatenate(
            [
                wq[:, q0 : q0 + 128],
                wq[:, 1024 + q0 : 1024 + q0 + 128],
                wq[:, q0 + 128 : q0 + 256],
                wq[:, 1024 + q0 + 128 : 1024 + q0 + 256],
            ],
            axis=1,
        )
        xt4 = np.ascontiguousarray(
            xb[b].T.reshape(8, 128, 4, 512).transpose(2, 1, 0, 3)
        )
        wv_c = wq[:, 2048 + q0 : 2048 + q0 + 256]
        in_maps.append(
            {
                "xT": xt4,
                "w_qk": np.ascontiguousarray(
                    wqk_c.reshape(8, 128, 512).transpose(1, 0, 2)
                ),
                "w_v": np.ascontiguousarray(
                    wv_c.reshape(8, 128, 256).transpose(1, 0, 2)
                ),
                "w_o": np.ascontiguousarray(
                    wo[q0 : q0 + 256, :].reshape(2, 128, 1024).transpose(1, 0, 2)
                ),
                "cos2": cos2,
                "sin2": sin2,
            }
        )
    return in_maps


def combine_outputs(results, b_out):
    out = np.empty((B, T, D), dtype=np.float32)
    for b in range(B):
        acc = results[4 * b]["out"].astype(np.float32)
        for c in range(4 * b + 1, 4 * b + 4):
            acc = acc + results[c]["out"].astype(np.float32)
        out[b] = acc + b_out[None, :]
    return out


def kernel(x, w_qkv, w_out, b_out, _trace=False, _tag=[0]):
    from concourse import bass_utils

    nc = _get_module()
    in_maps = make_in_maps(
        np.asarray(x, dtype=np.float32),
        np.asarray(w_qkv, dtype=np.float32),
        np.asarray(w_out, dtype=np.float32),
    )
    res = bass_utils.run_bass_kernel_spmd(
        nc, in_maps, core_ids=list(range(NCORES)), trace=_trace
    )
    if _trace:
        _CACHE["last_result"] = res
    return combine_outputs(res.results, np.asarray(b_out, dtype=np.float32))


# revision 7
# speedup vs baseline: 1.0592x; 1.0036x over previous
"""Multi-head self-attention (RoPE, eval-mode) Trainium2 Bass kernel.

Problem: B=2, T=2048, D=1024, H=16, d_head=64, fp32 I/O.

Sharding (8 cores): core c handles batch b=c//4 and the 4 heads
[4g, 4g+4) where g=c%4.  QKV/attention are head-local; the output
projection produces a per-core partial (contraction over this core's
256 head-dims) which the host sums across the 4 cores of each batch
and adds b_out.

Per-core design notes:
  - All matmul operands are bf16 (PSUM accumulation stays fp32): same
    PE rate as fp32r but halves input DMA and SBUF footprint, and
    LDWEIGHTS drops to ~95ns.  Measured rel-err ~7e-3 (gate: 2e-2).
  - q,k are computed feature-major (d_head on partitions, T on free);
    2 heads stacked per 128-partition tile.
  - RoPE: rotate_half is a 32-partition block swap (SBUF->SBUF DMAs)
    with the sign folded into the host-provided sin table; one mul on
    DVE, one on GpSimd so DVE doesn't bottleneck the QKV phase.
  - k is roped into per-head zero-padded kpad tiles (all-K=128 matmuls
    keep HAM from throttling the PE clock).
  - v is stored per head as [ones | v] 128-wide stationary tiles, so
    each PV matmul yields the softmax denominators (partitions 0:64)
    and attn^T (64:128) in one pass.
  - softmax skips max-subtraction and normalizes after PV with the
    fast DVE reciprocal.
  - attention is exp-bound (~1.11us per tk tile on the ACT engine);
    outproj matmuls are spread ONE per tk so per-tk PE work stays just
    under the exp time and the ACT engine never waits.
  - DMA priority: the first fm chain's deps (w_qk + x quarter 0) are
    issued first, chunked per 128-feature block so the PE starts ~9us
    in; later x quarters/wv/cos/sin/w_o trail behind.
  - trailing outproj units (last tq block) use the score-PSUM pool
    (free by then) and DMA out via sync/scalar queues so the gpsimd
    drain at teardown has nothing to wait on.
"""

import numpy as np
import ml_dtypes

B, T, D = 2, 2048, 1024
H = 16
DH = 64
NCORES = 8
P = 128

BF16 = ml_dtypes.bfloat16

_CACHE = {}


def _rope_tables_np():
    theta = 1.0 / (10000.0 ** (np.arange(0, DH, 2, dtype=np.float64) / DH))
    angles = np.outer(np.arange(T, dtype=np.float64), theta)  # (T, 32)
    angles = np.concatenate([angles, angles], axis=-1)  # (T, DH)
    cos = np.cos(angles).astype(np.float32)
    sin = np.sin(angles).astype(np.float32)
    cosT = np.ascontiguousarray(cos.T)  # (64, T)
    sinT = np.ascontiguousarray(sin.T)
    sinT_signed = np.concatenate([-sinT[0:32], sinT[32:64]], axis=0)
    cos2 = np.tile(cosT, (2, 1))  # (128, T)
    sin2 = np.tile(sinT_signed, (2, 1))
    return cos2.astype(BF16), sin2.astype(BF16)


def _build_module():
    import concourse.mybir as mybir
    import concourse.tile as tile
    from concourse import bacc

    f32 = mybir.dt.float32
    bf16 = mybir.dt.bfloat16

    nc = bacc.Bacc("TRN2", target_bir_lowering=False, debug=False)
    xT = nc.dram_tensor("xT", [4, P, 8, 512], bf16, kind="ExternalInput")
    w_qk = nc.dram_tensor("w_qk", [P, 8, 512], bf16, kind="ExternalInput")
    w_v = nc.dram_tensor("w_v", [P, 8, 256], bf16, kind="ExternalInput")
    w_o = nc.dram_tensor("w_o", [P, 2, 1024], bf16, kind="ExternalInput")
    cos2 = nc.dram_tensor("cos2", [P, T], bf16, kind="ExternalInput")
    sin2 = nc.dram_tensor("sin2", [P, T], bf16, kind="ExternalInput")
    out = nc.dram_tensor("out", [T, D], bf16, kind="ExternalOutput")

    Exp = mybir.ActivationFunctionType.Exp

    with tile.TileContext(nc) as tc:
        with tc.tile_pool(name="persist", bufs=1) as persist:
            wqk_sb = [
                persist.tile([P, 4, 512], bf16, tag=f"wqk{i}", name=f"wqk{i}")
                for i in range(2)
            ]
            wv_sb = persist.tile([P, 8, 256], bf16)
            q_q = [
                [
                    persist.tile([P, 512], bf16, tag=f"q{hp}_{q}", name=f"q{hp}_{q}")
                    for q in range(4)
                ]
                for hp in range(2)
            ]
            kpad = [
                [
                    [
                        persist.tile(
                            [P, 512], bf16, tag=f"kp{hp}{h}_{q}", name=f"kp{hp}{h}_{q}"
                        )
                        for q in range(4)
                    ]
                    for h in range(2)
                ]
                for hp in range(2)
            ]
            # per (tk-tile, head): [ones | v] stationary 128x128
            vaug = persist.tile([P, 16, 4, P], bf16)

            with (
                tc.tile_pool(name="attnsb", bufs=1) as apool,
                tc.tile_pool(name="expp", bufs=3) as epool,
                tc.tile_pool(name="norm", bufs=1) as npool,
            ):
                attn_q = [
                    [
                        apool.tile(
                            [P, 512], bf16, tag=f"at{hp}_{b}", name=f"at{hp}_{b}"
                        )
                        for b in range(4)
                    ]
                    for hp in range(2)
                ]

                with (
                    tc.tile_pool(name="xt", bufs=2) as xpool,
                    tc.tile_pool(name="kst", bufs=1) as kpool,
                    tc.tile_pool(name="qkv_ps", bufs=2, space="PSUM") as qkps,
                    tc.tile_pool(name="rope", bufs=2) as rpool,
                ):
                    cos_sb = kpool.tile([P, T], bf16)
                    sin_sb = kpool.tile([P, T], bf16)
                    kstack = [
                        [
                            kpool.tile(
                                [P, 512], bf16, tag=f"ks{hp}_{q}", name=f"ks{hp}_{q}"
                            )
                            for q in range(4)
                        ]
                        for hp in range(2)
                    ]

                    # ---- input DMAs, priority-ordered ----------------------
                    # First fm chain needs wqk chunk j + x chunk j in dc
                    # order; chunked DMAs let the chain start after ~256KB
                    # instead of 2MB.  Scalar queue: weights; sync: x.
                    xts = []
                    for tq in range(4):
                        xtl = xpool.tile([P, 4, 512], bf16, tag="xtl", name="xtl")
                        xth = xpool.tile([P, 4, 512], bf16, tag="xth", name="xth")
                        xts.append((xtl, xth))
                    # Whole-tile first-quarter DMAs: chunking starts the PE
                    # ~3us earlier but the mid-chain stalls reset the p-state
                    # ramp and cost more than they save.
                    nc.scalar.dma_start(wqk_sb[0][:], w_qk[:, 0:4, :])
                    nc.sync.dma_start(xts[0][0][:], xT[0, :, 0:4, :])
                    nc.scalar.dma_start(wqk_sb[1][:], w_qk[:, 4:8, :])
                    nc.sync.dma_start(xts[0][1][:], xT[0, :, 4:8, :])
                    for tq in range(1, 4):
                        nc.sync.dma_start(xts[tq][0][:], xT[tq, :, 0:4, :])
                        nc.sync.dma_start(xts[tq][1][:], xT[tq, :, 4:8, :])
                    nc.scalar.dma_start(wv_sb[:], w_v[:])
                    nc.scalar.dma_start(cos_sb[:], cos2[:])
                    nc.scalar.dma_start(sin_sb[:], sin2[:])

                    # Memsets early: no deps, run during the DMA window.
                    for hp in range(2):
                        for q in range(4):
                            nc.vector.memset(
                                kpad[hp][0][q][64:128, :].bitcast(f32), 0.0
                            )
                            nc.vector.memset(
                                kpad[hp][1][q][0:64, :].bitcast(f32), 0.0
                            )
                    # bf16 1.0 pair as an f32 bit pattern (0x3F803F80).
                    ones_f32 = float(np.uint32(0x3F803F80).view(np.float32))
                    nc.vector.memset(
                        vaug[:, :, :, 0:64].bitcast(f32), ones_f32
                    )

                    def fm_dst(cc, tq):
                        return (q_q if cc in (0, 2) else kstack)[cc // 2][tq]

                    def fm_chain(xt, tq, cc):
                        """One feature-major QKV chain (q or stacked k)."""
                        ps = qkps.tile([P, 512], f32, tag="fm", name="fmps")
                        for dc in range(8):
                            nc.tensor.matmul(
                                ps[:],
                                lhsT=wqk_sb[dc // 4][:, dc % 4, cc * P : (cc + 1) * P],
                                rhs=xt[dc // 4][:, dc % 4, :],
                                start=(dc == 0),
                                stop=(dc == 7),
                            )
                        # PSUM->SBUF copy on ACT (idle during QKV); DVE is
                        # busy with rope muls/adds.
                        nc.scalar.copy(fm_dst(cc, tq)[:], ps[:])

                    def v_chain(xt, tq, t4):
                        psv = qkps.tile([P, 256], f32, tag="v", name="vps")
                        for dc in range(8):
                            nc.tensor.matmul(
                                psv[:],
                                lhsT=xt[dc // 4][:, dc % 4, t4 * P : (t4 + 1) * P],
                                rhs=wv_sb[:, dc, :],
                                start=(dc == 0),
                                stop=(dc == 7),
                            )
                        tki = tq * 4 + t4
                        nc.scalar.copy(
                            vaug[:, tki, :, 64:128],
                            psv.rearrange("p (h e) -> p h e", e=64),
                        )

                    def rope_q(cc, qtr):
                        """RoPE one T-quarter of one q/k tensor."""
                        base = fm_dst(cc, qtr)
                        hs = slice(qtr * 512, (qtr + 1) * 512)
                        rot = rpool.tile([P, 512], bf16, tag="rot", name="rot")
                        for blk in range(4):
                            s = (blk ^ 1) * 32
                            eng = nc.sync if blk % 2 == 0 else nc.gpsimd
                            eng.dma_start(
                                rot[blk * 32 : (blk + 1) * 32, :],
                                base[s : s + 32, :],
                            )
                        t1 = rpool.tile([P, 512], bf16, tag="t1", name="t1")
                        nc.vector.tensor_mul(t1[:], base[:], cos_sb[:, hs])
                        nc.vector.tensor_mul(rot[:], rot[:], sin_sb[:, hs])
                        if cc in (0, 2):
                            nc.vector.tensor_add(base[:], t1[:], rot[:])
                        else:
                            hp = cc // 2
                            nc.vector.tensor_add(
                                kpad[hp][0][qtr][0:64, :], t1[0:64, :], rot[0:64, :]
                            )
                            nc.vector.tensor_add(
                                kpad[hp][1][qtr][64:128, :],
                                t1[64:128, :],
                                rot[64:128, :],
                            )

                    for tq in range(4):
                        for cc in (1, 0, 3, 2):
                            fm_chain(xts[tq], tq, cc)
                        for t4 in range(4):
                            v_chain(xts[tq], tq, t4)
                        for cc in (1, 0, 3, 2):
                            rope_q(cc, tq)

                # ---- attention + interleaved output projection ----------
                # PSUM: sc 4 + pv 2 + po 2 = 8 banks.
                with (
                    tc.tile_pool(name="wop", bufs=1) as wpool,
                    tc.tile_pool(name="ob", bufs=3) as opool,
                    tc.tile_pool(name="sc_ps", bufs=2, space="PSUM") as scps,
                    tc.tile_pool(name="pv_ps", bufs=1, space="PSUM") as pvps,
                    tc.tile_pool(name="po_ps", bufs=2, space="PSUM") as pops,
                ):
                    wo_sb = wpool.tile([P, 2, 1024], bf16)
                    nc.scalar.dma_start(wo_sb[:], w_o[:])

                    # Interleaved outproj, ONE matmul per tk: step s of 16
                    # emits (tqc, d2, hp) = (s//4, (s%4)//2, s%2) for block b.
                    # po tile [128,512] per (tqc,d2); cast+DMA after hp=1.
                    po_cur = [None]

                    def outproj_step(b, s):
                        tqc, d2, hp = s // 4, (s % 4) // 2, s % 2
                        row = b * 4 + tqc
                        if hp == 0:
                            po_cur[0] = pops.tile([P, 512], f32, tag="po", name="po")
                        po = po_cur[0]
                        nc.tensor.matmul(
                            po[:],
                            lhsT=attn_q[hp][b][:, tqc * P : (tqc + 1) * P],
                            rhs=wo_sb[:, hp, d2 * 512 : (d2 + 1) * 512],
                            start=(hp == 0),
                            stop=(hp == 1),
                        )
                        if hp == 1:
                            ob = opool.tile([P, 512], bf16, tag="ob", name="ob")
                            nc.vector.tensor_copy(ob[:], po[:])
                            seng = nc.sync if d2 == 0 else nc.gpsimd
                            seng.dma_start(
                                out[row * P : (row + 1) * P, d2 * 512 : (d2 + 1) * 512],
                                ob[:],
                            )

                    def outproj_unit_tail(b, tqc):
                        """Trailing unit: po lives in the (now free) score
                        PSUM pool; one cast + one DMA; sync/scalar queues so
                        the gpsimd teardown drain has nothing to wait on."""
                        row = b * 4 + tqc
                        po = scps.tile([P, 1024], f32, tag="sc", name="sc")
                        for d2 in range(2):
                            for hp in range(2):
                                nc.tensor.matmul(
                                    po[:, d2 * 512 : (d2 + 1) * 512],
                                    lhsT=attn_q[hp][b][:, tqc * P : (tqc + 1) * P],
                                    rhs=wo_sb[:, hp, d2 * 512 : (d2 + 1) * 512],
                                    start=(hp == 0),
                                    stop=(hp == 1),
                                )
                        ob = opool.tile([P, 1024], bf16, tag="obt", name="obt")
                        # split the cast: ACT is idle at the tail, and halving
                        # the vector cast shortens the unit-to-unit chain.
                        nc.scalar.copy(ob[:, 0:512], po[:, 0:512])
                        nc.vector.tensor_copy(ob[:, 512:1024], po[:, 512:1024])
                        eng = nc.sync if tqc % 2 == 0 else nc.scalar
                        eng.dma_start(out[row * P : (row + 1) * P, :], ob[:])

                    for hp in range(2):
                        for tq in range(4):  # tq blocks of 512
                            prev_b = tq - 1 if (hp == 1 and tq > 0) else None
                            pv = [
                                pvps.tile([P, 512], f32, tag=f"pv{h}", name=f"pv{h}")
                                for h in range(2)
                            ]
                            for tk in range(16):
                                if prev_b is not None:
                                    outproj_step(prev_b, tk)
                                sc = scps.tile([P, 1024], f32, tag="sc", name="sc")
                                ko = (tk % 4) * P
                                for h in range(2):
                                    nc.tensor.matmul(
                                        sc[:, h * 512 : (h + 1) * 512],
                                        lhsT=kpad[hp][h][tk // 4][:, ko : ko + P],
                                        rhs=q_q[hp][tq][:],
                                        start=True,
                                        stop=True,
                                    )
                                ex = epool.tile([P, 1024], bf16, tag="e", name="e")
                                nc.scalar.activation(ex[:], sc[:], Exp, scale=0.125)
                                for h in range(2):
                                    nc.tensor.matmul(
                                        pv[h][:],
                                        lhsT=vaug[:, tk, hp * 2 + h, :],
                                        rhs=ex[:, h * 512 : (h + 1) * 512],
                                        start=(tk == 0),
                                        stop=(tk == 15),
                                    )
                            for h in range(2):
                                rc = npool.tile([64, 512], f32, tag="rc", name="rc")
                                nc.vector.reciprocal_approx_fast(
                                    rc[:], pv[h][0:64, :]
                                )
                                hb = h * 64
                                nc.vector.tensor_mul(
                                    attn_q[hp][tq][hb : hb + 64, :],
                                    pv[h][64:128, :],
                                    rc[:],
                                )
                    for tqc in range(4):
                        outproj_unit_tail(3, tqc)

    nc.compile()
    return nc


def _get_module():
    if "nc" not in _CACHE:
        _CACHE["nc"] = _build_module()
    return _CACHE["nc"]


def make_in_maps(x, w_qkv, w_out):
    cos2, sin2 = _rope_tables_np()
    xb = x.astype(BF16)
    wq = w_qkv.astype(BF16)
    wo = w_out.astype(BF16)
    in_maps = []
    for c in range(NCORES):
        b, g = divmod(c, 4)
        q0 = 256 * g
        # column chunks: [q_hp0 | k_hp0 | q_hp1 | k_hp1]
        wqk_c = np.concatenate(
            [
                wq[:, q0 : q0 + 128],
                wq[:, 1024 + q0 : 1024 + q0 + 128],
                wq[:, q0 + 128 : q0 + 256],
                wq[:, 1024 + q0 + 128 : 1024 + q0 + 256],
            ],
            axis=1,
        )
        xt4 = np.ascontiguousarray(
            xb[b].T.reshape(8, 128, 4, 512).transpose(2, 1, 0, 3)
        )
        wv_c = wq[:, 2048 + q0 : 2048 + q0 + 256]
        in_maps.append(
            {
                "xT": xt4,
                "w_qk": np.ascontiguousarray(
                    wqk_c.reshape(8, 128, 512).transpose(1, 0, 2)
                ),
                "w_v": np.ascontiguousarray(
                    wv_c.reshape(8, 128, 256).transpose(1, 0, 2)
                ),
                "w_o": np.ascontiguousarray(
                    wo[q0 : q0 + 256, :].reshape(2, 128, 1024).transpose(1, 0, 2)
                ),
                "cos2": cos2,
                "sin2": sin2,
            }
        )
    return in_maps


def combine_outputs(results, b_out):
    out = np.empty((B, T, D), dtype=np.float32)
    for b in range(B):
        acc = results[4 * b]["out"].astype(np.float32)
        for c in range(4 * b + 1, 4 * b + 4):
            acc = acc + results[c]["out"].astype(np.float32)
        out[b] = acc + b_out[None, :]
    return out


def kernel(x, w_qkv, w_out, b_out, _trace=False, _tag=[0]):
    from concourse import bass_utils

    nc = _get_module()
    in_maps = make_in_maps(
        np.asarray(x, dtype=np.float32),
        np.asarray(w_qkv, dtype=np.float32),
        np.asarray(w_out, dtype=np.float32),
    )
    res = bass_utils.run_bass_kernel_spmd(
        nc, in_maps, core_ids=list(range(NCORES)), trace=_trace
    )
    if _trace:
        _CACHE["last_result"] = res
    return combine_outputs(res.results, np.asarray(b_out, dtype=np.float32))
